# revision 13
# baseline (speedup 1.0000x reference)
"""AttentionPooling Trainium2 kernel.

Problem: segment-softmax attention pooling over N=500000 nodes, H=256 features,
G=2048 graphs (sorted segment ids):
    h      = relu(x @ gate_w1 + gate_b1)            [N, 128]
    s      = (h @ gate_w2 + gate_b2)[:, 0]          [N]
    alpha  = segment_softmax(s, batch)              [N]
    feat   = relu(x @ feat_w + feat_b)              [N, 256]
    emb    = segment_sum(alpha[:, None] * feat)     [G, 256]
returns (emb, alpha).

Strategy (graph-level data parallel over 8 cores):
  * batch is sorted, so shard graphs contiguously: core c owns graphs
    [256c, 256(c+1)) and therefore a contiguous node range. No collectives.
  * softmax max-subtraction is skipped (scores are O(1); exp never overflows;
    alpha is mathematically unchanged).
  * Single pass over x per core, nodes on partitions ("orientation A"):
      - PE: [h | feat] = xT_tile.T @ [gate_w1 | feat_w]  (bf16, fp32 accum),
        plus a K=1 ones-row matmul that adds the biases into PSUM,
        plus a one-hot matmul that segment-sums e_n * feat_n into U.
      - DVE: scores via scalar_tensor_tensor(max(h_psum,0) * w2, accum_out),
        e-scaled one-hot rows via tensor_scalar((iota==bid) * e).
      - ACT: e = exp(s + b2); feat relu PSUM->SBUF (bf16 out).
    Denominators ride as a 257th ones-column of the pool matmul rhs.
  * Host finishes with the O(G*H + N) division/gather: emb = U/den and
    alpha = e / den[batch] (pure unshard-time normalization).
"""

import math
import os

import ml_dtypes
import numpy as np

import concourse.bass as bass
import concourse.mybir as mybir
from concourse import bacc
import concourse.tile as tile
from concourse.bass_utils import run_bass_kernel_spmd

P = 128
H = 256
HF = 384          # h (128) + feat (256) fused output columns
G = 2048
NCORES = 8
GL = G // NCORES  # graphs per core
NB = 2048         # nodes per DMA macro block
TPM = NB // P     # node tiles per macro block

BF16 = mybir.dt.bfloat16
F32 = mybir.dt.float32
BF16_NP = ml_dtypes.bfloat16

LAST_RESULT = None  # BassKernelResults of the most recent run (for test.py)


def _build(n_tiles: int, tile_chunks: list[tuple[int, ...]], b2: float) -> bass.Bass:
    nc = bacc.Bacc()
    n_pad = n_tiles * P
    n_macros = n_tiles // TPM

    xt_d = nc.dram_tensor("xt", [H, n_pad], BF16, kind="ExternalInput")
    # Constants are packed into two blocks so each lands in SBUF via ONE DMA
    # (instruction sync-wait slots are scarce; scattered const DMAs put too
    # many semaphore waits on their first consumers).
    # cbf cols: [0:384]=wcat0, [384:768]=wcat1, [768:1024]=iota,
    #           row0 only: [1024:1152]=ones, [1152:1536]=bcat
    cbf_d = nc.dram_tensor("cbf", [P, 1536], BF16, kind="ExternalInput")
    # cf32 cols: [0:128]=w2b broadcast, [128:128+n_tiles]=bids, [last]=b2
    cf32_d = nc.dram_tensor("cf32", [P, P + n_tiles + 1], F32, kind="ExternalInput")

    e_out_d = nc.dram_tensor("e_out", [P, n_tiles], F32, kind="ExternalOutput")
    u_out_d = nc.dram_tensor("u_out", [2 * P, H + 1], F32, kind="ExternalOutput")

    first_use: dict[int, int] = {}
    last_use: dict[int, int] = {}
    for t, chs in enumerate(tile_chunks):
        for ch in chs:
            first_use.setdefault(ch, t)
            last_use[ch] = t

    relu = mybir.ActivationFunctionType.Relu
    expf = mybir.ActivationFunctionType.Exp
    op_max = mybir.AluOpType.max
    op_mult = mybir.AluOpType.mult
    op_iseq = mybir.AluOpType.is_equal

    with tile.TileContext(nc) as tc:
        with (
            tc.tile_pool(name="const", bufs=1) as constp,
            tc.tile_pool(name="xt", bufs=3) as xtp,
            tc.tile_pool(name="work", bufs=4) as workp,
            tc.tile_pool(name="small", bufs=4) as smallp,
            tc.tile_pool(name="hf", bufs=3, space="PSUM") as hfp,
            tc.tile_pool(name="upsum", bufs=1, space="PSUM") as upp,
        ):
            cbf = constp.tile([P, 1536], BF16, tag="cbf")
            nc.sync.dma_start(cbf[:], cbf_d[:, :])
            cf32 = constp.tile([P, P + n_tiles + 1], F32, tag="cf32")
            nc.sync.dma_start(cf32[:], cf32_d[:, :])
            wcat0 = cbf[:, 0:HF]
            wcat1 = cbf[:, HF : 2 * HF]
            iota = cbf[:, 2 * HF : 2 * HF + 2 * P]
            onesb = cbf[0:1, 1024 : 1024 + P]
            bcat = cbf[0:1, 1152 : 1152 + HF]
            w2b = cf32[:, 0:P]
            bids = cf32[:, P : P + n_tiles]
            b2t = cf32[:, P + n_tiles : P + n_tiles + 1]
            e_all = constp.tile([P, n_tiles], F32, tag="e_all")
            sc = constp.tile([P, P], BF16, tag="sc")

            # Pre-join the const DMA lanes into the DVE/ACT engine clocks so
            # steady-state instructions need at most ONE sync wait (the
            # S2S2D2 TensorScalarPtr format has a single wait slot).
            joinv = constp.tile([P, 1], F32, tag="joinv")
            nc.vector.tensor_copy(joinv[:], cf32[:, 0:1])
            nc.vector.tensor_copy(joinv[:], cbf[:, 0:1])
            joina = constp.tile([P, 1], F32, tag="joina")
            nc.scalar.copy(joina[:], cf32[:, 0:1])

            upsum = {
                ch: upp.tile([P, H + 1], F32, tag=f"U{ch}", name=f"U{ch}")
                for ch in sorted(first_use)
            }

            for m in range(n_macros):
                xt0 = xtp.tile([P, NB], BF16, tag="xt0")
                nc.sync.dma_start(xt0[:], xt_d[0:P, m * NB : (m + 1) * NB])
                xt1 = xtp.tile([P, NB], BF16, tag="xt1")
                nc.sync.dma_start(xt1[:], xt_d[P:H, m * NB : (m + 1) * NB])
                for tt in range(TPM):
                    t = m * TPM + tt
                    sl = slice(tt * P, (tt + 1) * P)
                    hf = hfp.tile([P, HF], F32, tag="hf")
                    nc.tensor.matmul(hf[:], lhsT=xt0[:, sl], rhs=wcat0[:], start=True, stop=False)
                    nc.tensor.matmul(hf[:], lhsT=xt1[:, sl], rhs=wcat1[:], start=False, stop=False)
                    nc.tensor.matmul(hf[:], lhsT=onesb[:], rhs=bcat[:], start=False, stop=True)

                    s_t = smallp.tile([P, 1], F32, tag="s")
                    nc.vector.scalar_tensor_tensor(
                        out=sc[:], in0=hf[:, 0:P], scalar=0.0, in1=w2b[:],
                        op0=op_max, op1=op_mult, accum_out=s_t[:],
                    )
                    nc.scalar.activation(e_all[:, t : t + 1], s_t[:], expf, bias=b2t[:])

                    featr = workp.tile([P, H + 1], BF16, tag="featr")
                    nc.gpsimd.memset(featr[:, H : H + 1], 1.0)
                    nc.scalar.activation(featr[:, 0:H], hf[:, P:HF], relu)

                    chs = tile_chunks[t]
                    if chs:
                        eoh = workp.tile([P, 2 * P], BF16, tag="eoh")
                        nc.vector.tensor_scalar(
                            out=eoh[:], in0=iota[:],
                            scalar1=bids[:, t : t + 1], scalar2=e_all[:, t : t + 1],
                            op0=op_iseq, op1=op_mult,
                        )
                        for ch in chs:
                            nc.tensor.matmul(
                                upsum[ch][:],
                                lhsT=eoh[:, ch * P : (ch + 1) * P],
                                rhs=featr[:],
                                start=(first_use[ch] == t),
                                stop=(last_use[ch] == t),
                                skip_group_check=True,
                            )

            for ch in (0, 1):
                u_sb = constp.tile([P, H + 1], F32, tag=f"usb{ch}")
                if ch in upsum:
                    nc.vector.tensor_copy(u_sb[:], upsum[ch][:])
                else:
                    nc.vector.memset(u_sb[:], 0.0)
                nc.sync.dma_start(u_out_d[ch * P : (ch + 1) * P, :], u_sb[:])
            nc.sync.dma_start(e_out_d[:, :], e_all[:])

    nc.compile()
    return nc


def kernel(x, batch, gate_w1, gate_b1, gate_w2, gate_b2, feat_w, feat_b):
    global LAST_RESULT
    x = np.asarray(x, dtype=np.float32)
    batch = np.asarray(batch, dtype=np.int64)
    gate_w1 = np.asarray(gate_w1, dtype=np.float32)
    gate_b1 = np.asarray(gate_b1, dtype=np.float32)
    gate_w2 = np.asarray(gate_w2, dtype=np.float32)
    gate_b2 = np.asarray(gate_b2, dtype=np.float32)
    feat_w = np.asarray(feat_w, dtype=np.float32)
    feat_b = np.asarray(feat_b, dtype=np.float32)
    n = x.shape[0]

    bounds = np.searchsorted(batch, np.arange(0, G + 1, GL)).astype(np.int64)
    counts = np.diff(bounds)
    n_tiles = max(1, math.ceil(int(counts.max()) / P))
    n_tiles = math.ceil(n_tiles / TPM) * TPM
    n_pad = n_tiles * P

    x_bf = x.astype(BF16_NP)
    in_maps = []
    chunk_sets = [set() for _ in range(n_tiles)]
    core_meta = []
    for c in range(NCORES):
        s, e = int(bounds[c]), int(bounds[c + 1])
        cnt = e - s
        xt = np.zeros((H, n_pad), dtype=BF16_NP)
        xt[:, :cnt] = x_bf[s:e].T
        bid = np.full(n_pad, 300.0, dtype=np.float32)
        bid[:cnt] = (batch[s:e] - c * GL).astype(np.float32)
        bids = np.ascontiguousarray(bid.reshape(n_tiles, P).T).astype(np.float32)
        for t in range(n_tiles):
            ids = bid[t * P : (t + 1) * P]
            real = ids < 2 * P
            if real.any():
                lo = int(ids[real].min()) // P
                hi = int(ids[real].max()) // P
                chunk_sets[t].update(range(lo, hi + 1))
        core_meta.append((s, e, cnt))
        in_maps.append({"xt": xt, "bids": bids})

    wcat = np.concatenate([gate_w1, feat_w], axis=1).astype(BF16_NP)
    cbf = np.zeros((P, 1536), dtype=BF16_NP)
    cbf[:, 0:HF] = wcat[0:P]
    cbf[:, HF : 2 * HF] = wcat[P:H]
    cbf[:, 2 * HF : 2 * HF + 2 * P] = np.arange(2 * P, dtype=np.float32)[None, :]
    cbf[0, 1024 : 1024 + P] = 1.0
    cbf[0, 1152 : 1152 + HF] = np.concatenate([gate_b1, feat_b]).astype(BF16_NP)
    for m in in_maps:
        cf32 = np.empty((P, P + n_tiles + 1), dtype=np.float32)
        cf32[:, 0:P] = gate_w2[:, 0][None, :]
        cf32[:, P : P + n_tiles] = m.pop("bids")
        cf32[:, P + n_tiles] = float(gate_b2[0])
        m.update(cbf=cbf, cf32=cf32)

    tile_chunks = [tuple(sorted(cs)) for cs in chunk_sets]
    nc = _build(n_tiles, tile_chunks, float(gate_b2[0]))

    trace = bool(int(os.environ.get("KERNEL_TRACE", "0")))
    LAST_RESULT = run_bass_kernel_spmd(
        nc, in_maps, core_ids=list(range(NCORES)), trace=trace
    )
    results = LAST_RESULT.results

    emb = np.empty((G, H), dtype=np.float32)
    den = np.empty(G, dtype=np.float32)
    alpha = np.empty(n, dtype=np.float32)
    for c in range(NCORES):
        u = results[c]["u_out"]
        den_c = u[:, H]
        emb[c * GL : (c + 1) * GL] = u[:, :H] / np.maximum(den_c, 1e-30)[:, None]
        den[c * GL : (c + 1) * GL] = den_c
        s, e, cnt = core_meta[c]
        e_vals = results[c]["e_out"].T.reshape(-1)[:cnt]
        alpha[s:e] = e_vals / np.maximum(den[batch[s:e]], 1e-30)
    return emb, alpha
